# revision 1
# baseline (speedup 1.0000x reference)
"""Trainium2 Bass kernel v5: adapter block, data-parallel over 8 cores.

v3 vs v2 (engine rebalance, stall reduction):
  - reps run inside one tc.For_i hardware loop (body = one 32-tile pass,
    row offsets repeat per rep), so the NEFF stays small for any reps —
    instruction-stream traffic no longer scales with reps.
  - GPSIMD cannot touch PSUM, so the final combine is split: Z cols via DVE
    scalar_tensor_tensor from PSUM, the rest via ACT copy*rstd (PSUM->SBUF)
    then GPSIMD add; the xt evac is split ACT/DVE to balance.
  - relu on DVE, (mean,std) evac on ACT into a base-0 msrow tile.
  - wd/cb carry a 17th column (zeros / (0,1)) so psi row 16 accumulates to
    std; one DVE relu over [17,128] yields relu rows + std row (std>0), and
    the K=17 up matmul folds the std*b_up bias via wu_ext row 16.
  - stats transpose lands in cols 128:256 of the psi PSUM tile (saves a
    bank); pup double-buffered with the freed banks (2+2+4 = 8 banks).
"""

import numpy as np

D_MODEL = 768
BOTTLENECK = 16
SCALE = 0.1
LN_EPS = 1e-5
P = 128
N_CORES = 8
CHUNKS = D_MODEL // P  # 6

XT_BUFS = 2
PSI_BUFS = 2
UP_BUFS = 2

# Final-combine: cols [0:Z) DVE stt; cols [Z:768) ACT copy*rstd + Pool add.
Z_DVE = 64
# xt evacuation: cols [0:EA) on ACT, cols [EA:768) on DVE.
EA_ACT = 576
# (mean,std) evac engine: True -> ACT, False -> DVE.
MSROW_ACT = True

SKEW = 0
SKEW_C = 1

_CACHE: dict = {}


def _build(rows_per_core: int, reps: int = 1):
    from contextlib import ExitStack

    import concourse.bacc as bacc
    import concourse.tile as tile
    from concourse import mybir

    nc = bacc.Bacc(
        "TRN2",
        target_bir_lowering=False,
        debug=False,
        enable_asserts=False,
        num_devices=N_CORES,
    )
    f32 = mybir.dt.float32
    bf16 = mybir.dt.bfloat16

    x_d = nc.dram_tensor("x", [rows_per_core, D_MODEL], bf16, kind="ExternalInput").ap()
    wd_d = nc.dram_tensor(
        "wd", [D_MODEL, BOTTLENECK + 1], bf16, kind="ExternalInput"
    ).ap()
    cb_d = nc.dram_tensor("cb", [2, BOTTLENECK + 1], bf16, kind="ExternalInput").ap()
    wu_d = nc.dram_tensor(
        "wu", [BOTTLENECK + 1, D_MODEL], bf16, kind="ExternalInput"
    ).ap()
    id16_d = nc.dram_tensor("id16", [P, P], bf16, kind="ExternalInput").ap()
    id32_d = nc.dram_tensor("id32", [P, P], f32, kind="ExternalInput").ap()
    out_d = nc.dram_tensor(
        "out", [rows_per_core, D_MODEL], bf16, kind="ExternalOutput"
    ).ap()

    ntiles = rows_per_core // P
    Sqrt = mybir.ActivationFunctionType.Sqrt
    Copy = mybir.ActivationFunctionType.Copy
    mult = mybir.AluOpType.mult
    add = mybir.AluOpType.add

    with tile.TileContext(nc) as tc, ExitStack() as ctx:
        consts = ctx.enter_context(tc.tile_pool(name="consts", bufs=1))
        xpool = ctx.enter_context(tc.tile_pool(name="xpool", bufs=6))
        opool = ctx.enter_context(tc.tile_pool(name="opool", bufs=4))
        xtpool = ctx.enter_context(tc.tile_pool(name="xtpool", bufs=4))
        repool = ctx.enter_context(tc.tile_pool(name="repool", bufs=4))
        uppool = ctx.enter_context(tc.tile_pool(name="uppool", bufs=3))
        tiny = ctx.enter_context(tc.tile_pool(name="tiny", bufs=8))
        p_xt = ctx.enter_context(tc.tile_pool(name="p_xt", bufs=XT_BUFS, space="PSUM"))
        p_psi = ctx.enter_context(
            tc.tile_pool(name="p_psi", bufs=PSI_BUFS, space="PSUM")
        )
        p_up = ctx.enter_context(tc.tile_pool(name="p_up", bufs=UP_BUFS, space="PSUM"))

        # ---- constants (loaded once; identities first, stage_a needs them) --
        id16_sb = consts.tile([P, P], bf16)
        nc.sync.dma_start(out=id16_sb, in_=id16_d)
        id32_sb = consts.tile([P, P], f32)
        nc.sync.dma_start(out=id32_sb, in_=id32_d)
        eps_sb = consts.tile([P, 1], f32)
        nc.vector.memset(eps_sb, LN_EPS)
        wd_sb = consts.tile([P, CHUNKS, BOTTLENECK + 1], bf16)
        cb_sb = consts.tile([2, BOTTLENECK + 1], bf16)
        wu_sb = consts.tile([BOTTLENECK + 1, D_MODEL], bf16)

        def load_b_consts():
            nc.sync.dma_start(out=wd_sb, in_=wd_d.rearrange("(c p) k -> p c k", p=P))
            nc.sync.dma_start(out=cb_sb, in_=cb_d)

        def load_c_consts():
            nc.sync.dma_start(out=wu_sb, in_=wu_d)

        pair_x: dict = {}

        def stage_a(i):
            """DMA-in, LN stats chain, PE transposes, evacs."""
            r0 = (i % ntiles) * P
            if i >= 2:
                if i % 2 == 0:
                    xpair = xpool.tile([P, 2, D_MODEL], bf16, tag="x")
                    rp = (i % ntiles) * P
                    nc.sync.dma_start(
                        out=xpair,
                        in_=x_d[rp : rp + 2 * P, :].rearrange("(a p) d -> p a d", a=2),
                    )
                    pair_x[i] = xpair
                    x_sb = xpair[:, 0, :]
                else:
                    x_sb = pair_x.pop(i - 1)[:, 1, :]
            else:
                x_sb = xpool.tile([P, D_MODEL], bf16, tag="x")
                nc.sync.dma_start(out=x_sb, in_=x_d[r0 : r0 + P, :])

            st6 = tiny.tile([P, 2, 6], f32, tag="st6")
            nc.vector.bn_stats(out=st6[:, 0, :], in_=x_sb[:, 0 : D_MODEL // 2])
            nc.vector.bn_stats(out=st6[:, 1, :], in_=x_sb[:, D_MODEL // 2 :])
            mv = tiny.tile([P, 2], f32, tag="mv")  # (mean, var) -> (mean, std)
            nc.vector.bn_aggr(out=mv, in_=st6)
            nc.scalar.activation(out=mv[:, 1:2], in_=mv[:, 1:2], func=Sqrt, bias=eps_sb)
            rstd = tiny.tile([P, 1], f32, tag="rstd")
            nc.vector.reciprocal(out=rstd, in_=mv[:, 1:2])

            # psi tile cols 128:256 (partitions 0:2) host the stats
            # transpose (transpose outputs must sit at PSUM partition 0).
            ppsi = p_psi.tile([BOTTLENECK + 1, 2 * P], f32, tag="psi")
            nc.tensor.transpose(out=ppsi[0:2, P : 2 * P], in_=mv, identity=id32_sb)
            pxt = p_xt.tile([P, D_MODEL], bf16, tag="pxt")
            for c in range(CHUNKS):
                nc.tensor.transpose(
                    out=pxt[:, c * P : (c + 1) * P],
                    in_=x_sb[:, c * P : (c + 1) * P],
                    identity=id16_sb,
                )
            msrow = tiny.tile([2, P], bf16, tag="msrow")
            if MSROW_ACT:
                nc.scalar.activation(
                    out=msrow, in_=ppsi[0:2, P : 2 * P], func=Copy
                )
            else:
                nc.vector.tensor_copy(out=msrow, in_=ppsi[0:2, P : 2 * P])
            xt_sb = xtpool.tile([P, D_MODEL], bf16, tag="xt")
            if EA_ACT > 0:
                nc.scalar.activation(
                    out=xt_sb[:, 0:EA_ACT], in_=pxt[:, 0:EA_ACT], func=Copy
                )
            if EA_ACT < D_MODEL:
                nc.vector.tensor_copy(out=xt_sb[:, EA_ACT:], in_=pxt[:, EA_ACT:])
            return dict(
                x_sb=x_sb, rstd=rstd, msrow=msrow, xt_sb=xt_sb, ppsi=ppsi, r0=r0
            )

        def stage_b(s):
            """down-proj matmuls + relu evac (DVE)."""
            ppsi = s["ppsi"]
            for c in range(CHUNKS):
                nc.tensor.matmul(
                    ppsi[:, 0:P],
                    lhsT=wd_sb[:, c, :],
                    rhs=s["xt_sb"][:, c * P : (c + 1) * P],
                    start=(c == 0),
                    stop=False,
                )
            # corrections: -c (x) mean + b' (x) std; col 16 of cb is (0,1) so
            # psi row 16 = std (rows of wd col 16 are zero).
            nc.tensor.matmul(
                ppsi[:, 0:P], lhsT=cb_sb, rhs=s["msrow"], start=False, stop=True
            )
            # relu over psi rows AND the std row: relu(std) = std since std>0.
            relu_ext = repool.tile([BOTTLENECK + 1, P], bf16, tag="relu")
            nc.vector.tensor_scalar_max(out=relu_ext, in0=ppsi[:, 0:P], scalar1=0.0)
            s["relu_ext"] = relu_ext

        pair_o: dict = {}

        def stage_c(s, i):
            """up-proj (bias folded in), final combine (GPSIMD), DMA-out."""
            pup = p_up.tile([P, D_MODEL], f32, tag="pup")
            for lo, hi in ((0, 512), (512, D_MODEL)):
                nc.tensor.matmul(
                    pup[:, lo:hi],
                    lhsT=s["relu_ext"],
                    rhs=wu_sb[:, lo:hi],
                    start=True,
                    stop=True,
                )
            if i % 2 == 0:
                pair_o[i] = opool.tile(
                    [P, 2, D_MODEL], bf16, tag="out", name=f"opair{i}"
                )
                out_sb = pair_o[i][:, 0, :]
            else:
                out_sb = pair_o[i - 1][:, 1, :]
            Z = Z_DVE
            if Z > 0:
                nc.vector.scalar_tensor_tensor(
                    out=out_sb[:, 0:Z],
                    in0=pup[:, 0:Z],
                    scalar=s["rstd"],
                    in1=s["x_sb"][:, 0:Z],
                    op0=mult,
                    op1=add,
                )
            up_sb = uppool.tile([P, D_MODEL - Z], bf16, tag="up")
            nc.scalar.activation(
                out=up_sb, in_=pup[:, Z:], func=Copy, scale=s["rstd"]
            )
            nc.gpsimd.tensor_add(out_sb[:, Z:], up_sb, s["x_sb"][:, Z:])
            r0 = s["r0"]
            if i % 2 == 1:
                rp = r0 - P
                nc.sync.dma_start(
                    out=out_d[rp : rp + 2 * P, :].rearrange("(a p) d -> p a d", a=2),
                    in_=pair_o.pop(i - 1),
                )

        load_b_consts()
        load_c_consts()

        def one_rep():
            live_a: list = []
            live_b: list = []
            n_c = 0
            for i in range(ntiles + SKEW + SKEW_C):
                if i < ntiles:
                    live_a.append(stage_a(i))
                if i >= SKEW and live_a:
                    s = live_a.pop(0)
                    stage_b(s)
                    live_b.append(s)
                if i >= SKEW + SKEW_C and live_b:
                    stage_c(live_b.pop(0), n_c)
                    n_c += 1

        if reps == 1:
            one_rep()
        else:
            with tc.For_i(0, reps, 1, hint_engines=(mybir.EngineType.PE,)):
                one_rep()

    nc.compile()
    return nc


def _get_nc(rows_per_core: int, reps: int = 1):
    key = (rows_per_core, reps)
    if key not in _CACHE:
        _CACHE[key] = _build(rows_per_core, reps)
    return _CACHE[key]


def _host_consts(ln_gamma, ln_beta, w_down, b_down, w_up, b_up):
    import ml_dtypes

    bf = ml_dtypes.bfloat16
    ln_gamma = np.asarray(ln_gamma, np.float32)
    ln_beta = np.asarray(ln_beta, np.float32)
    w_down = np.asarray(w_down, np.float32)
    b_down = np.asarray(b_down, np.float32)
    w_up = np.asarray(w_up, np.float32)
    b_up = np.asarray(b_up, np.float32)

    wd_eff = (ln_gamma[:, None] * w_down).astype(bf)
    b_eff = b_down + ln_beta @ w_down
    wd_ext = np.concatenate(
        [wd_eff.astype(np.float32), np.zeros((D_MODEL, 1), np.float32)], axis=1
    ).astype(bf)
    cb = np.stack(
        [
            np.concatenate([-wd_eff.astype(np.float32).sum(0), [0.0]]),
            np.concatenate([b_eff, [1.0]]),
        ]
    ).astype(bf)
    wu_ext = np.concatenate([SCALE * w_up, (SCALE * b_up)[None, :]], axis=0).astype(bf)
    return dict(
        wd=wd_ext,
        cb=np.ascontiguousarray(cb),
        wu=np.ascontiguousarray(wu_ext),
        id16=np.eye(P, dtype=bf),
        id32=np.eye(P, dtype=np.float32),
    )


def kernel(x, ln_gamma, ln_beta, w_down, b_down, w_up, b_up):
    import ml_dtypes

    from concourse.bass_utils import run_bass_kernel_spmd

    bf = ml_dtypes.bfloat16
    x = np.asarray(x)
    b, t, d = x.shape
    rows = b * t
    rpc = rows // N_CORES
    consts = _host_consts(ln_gamma, ln_beta, w_down, b_down, w_up, b_up)
    xf = np.ascontiguousarray(x.reshape(rows, d)).astype(bf)
    in_maps = [
        dict(x=np.ascontiguousarray(xf[i * rpc : (i + 1) * rpc]), **consts)
        for i in range(N_CORES)
    ]
    nc = _get_nc(rpc)
    res = run_bass_kernel_spmd(nc, in_maps, core_ids=list(range(N_CORES)))
    out = np.concatenate([r["out"] for r in res.results], axis=0)
    return np.ascontiguousarray(out.reshape(b, t, d).astype(np.float32))

